# revision 1
# baseline (speedup 1.0000x reference)
"""Trainium2 Bass kernel for CompositionModel (gnn_message_passing).

Model: per-cell MLP over [log1p(X) ++ Z[cell_to_batch]] followed by a
segment-mean over batch labels.

Strategy:
  * Host: sort cells by segment id, pad each segment run to a multiple of 64
    so every 64-cell "minichunk" is single-segment; gather Z rows per cell;
    ship everything transposed (features on partitions) in bf16, blocked as
    [P, 512]-column blocks; two blocks share one DMA/log1p pass.
  * Device (8 cores, data-parallel over cells, identical static program):
      log1p (ACT Ln, 1024 cols/op) -> L1 matmul (K=128 X-part + K=32 Z-part,
      bf16) -> bias+ReLU -> fp8 h1 -> L2 as fp8 DoubleRow matmuls against
      W2 split into a (hi, lo) fp8 pair sharing one x64 scale (W2 is then
      effectively exact; only h1 carries fp8 rounding, which averages out
      in the segment mean) -> fused bias+ReLU+cast on DVE -> GpSimd
      pre-folds each minichunk in half -> grouped DVE tensor_reduce.
      The third (linear) layer commutes with the segment sum and is applied
    on the host to the 512x256 segment sums instead of 500k cells.
  * Host epilogue: subtract the (identical, analytically known) contribution
    of pad cells, scatter-add minichunk sums into segment sums, undo the x64
    W2 scale, apply W3/b3 and divide by true counts.
"""

import numpy as np
import ml_dtypes

import concourse.bacc as bacc
import concourse.mybir as mybir
import concourse.tile as tile
from concourse.bass_utils import run_bass_kernel_spmd

BF16 = ml_dtypes.bfloat16
FP8 = ml_dtypes.float8_e4m3fn

N_CORES = 8
DX = 128
DZ = 32
H = 256
B = 512
MC = 64            # minichunk: cells per single-segment group
BLK = 512          # cells per device block (matmul moving free dim)
NBLK = 126         # blocks per core (fits the fixed reference input)
W2SCALE = 64.0     # fp8 pre-scale on W2/b2, divided out on the host

_compiled = {}
_last_in_maps = None


def _build_program(nblk):
    f32 = mybir.dt.float32
    bf16 = mybir.dt.bfloat16
    fp8 = mybir.dt.float8e4
    Alu = mybir.AluOpType
    Act = mybir.ActivationFunctionType
    DR = mybir.MatmulPerfMode.DoubleRow
    mc_per_core = nblk * (BLK // MC)

    nc = bacc.Bacc("TRN2", target_bir_lowering=False, debug=False,
                   num_devices=N_CORES)

    xt_d = nc.dram_tensor("xt", [nblk // 2, DX, 2 * BLK], bf16,
                          kind="ExternalInput")
    zct_d = nc.dram_tensor("zct", [nblk, DZ, BLK], bf16, kind="ExternalInput")
    w1x_d = nc.dram_tensor("w1x", [DX, H], bf16, kind="ExternalInput")
    w1z_d = nc.dram_tensor("w1z", [DZ, H], bf16, kind="ExternalInput")
    # [m-half][hi/lo][p, ktile*128] fp8, pre-scaled by W2SCALE
    w2_d = nc.dram_tensor("w2", [2, 2, 128, 2 * 128], fp8,
                          kind="ExternalInput")
    b1_d = nc.dram_tensor("b1", [2, 128, 1], f32, kind="ExternalInput")
    b2_d = nc.dram_tensor("b2", [2, 128, 1], f32, kind="ExternalInput")
    out_d = nc.dram_tensor("out", [128, 2 * mc_per_core], f32,
                           kind="ExternalOutput")

    with tile.TileContext(nc) as tc:
        with tc.tile_pool(name="consts", bufs=1) as cpool, \
             tc.tile_pool(name="work", bufs=4) as pool, \
             tc.tile_pool(name="psum", bufs=2, space="PSUM") as psum:

            w1xa = cpool.tile([DX, 128], bf16, tag="w1xa")
            w1xb = cpool.tile([DX, 128], bf16, tag="w1xb")
            nc.sync.dma_start(w1xa[:], w1x_d[:, 0:128])
            nc.sync.dma_start(w1xb[:], w1x_d[:, 128:256])
            w1za = cpool.tile([DZ, 128], bf16, tag="w1za")
            w1zb = cpool.tile([DZ, 128], bf16, tag="w1zb")
            nc.sync.dma_start(w1za[:], w1z_d[:, 0:128])
            nc.sync.dma_start(w1zb[:], w1z_d[:, 128:256])
            w2t = {}
            for m in range(2):
                for t in range(2):
                    w = cpool.tile([128, 2 * 128], fp8, tag=f"w2_{m}{t}")
                    nc.sync.dma_start(w[:], w2_d[m, t])
                    w2t[m, t] = w[:].rearrange("p (k m) -> p k m", k=2)
            b1a = cpool.tile([128, 1], f32, tag="b1a")
            b1b = cpool.tile([128, 1], f32, tag="b1b")
            b2a = cpool.tile([128, 1], f32, tag="b2a")
            b2b = cpool.tile([128, 1], f32, tag="b2b")
            nc.sync.dma_start(b1a[:], b1_d[0])
            nc.sync.dma_start(b1b[:], b1_d[1])
            nc.sync.dma_start(b2a[:], b2_d[0])
            nc.sync.dma_start(b2b[:], b2_d[1])
            ones = cpool.tile([128, 1], f32, tag="ones")
            nc.vector.memset(ones[:], 1.0)

            out2 = cpool.tile([128, 2 * mc_per_core], f32, tag="out2")

            # two blocks share one DMA + one Ln op (amortize ACT overhead);
            # the Ln is emitted two superblocks ahead so it fills ACT idle
            # time without ever delaying a relu that gates the PE
            def emit_ln(k):
                xt = pool.tile([DX, 2 * BLK], bf16, tag="xt")
                nc.sync.dma_start(xt[:], xt_d[k])
                xl = pool.tile([DX, 2 * BLK], bf16, tag="xl")
                nc.scalar.activation(xl[:], xt[:], Act.Ln, bias=ones[:])
                return xl

            nsb = nblk // 2
            xls_ahead = [emit_ln(0), emit_ln(1) if nsb > 1 else None]
            for sblk in range(nsb):
                xl_cur = xls_ahead.pop(0)
                for half in range(2):
                    blk = 2 * sblk + half
                    xls = xl_cur[:, half * BLK:(half + 1) * BLK]
                    zct = pool.tile([DZ, BLK], bf16, tag="zct")
                    nc.sync.dma_start(zct[:], zct_d[blk])

                    ps1a = psum.tile([128, BLK], f32, tag="ps1a")
                    nc.tensor.matmul(ps1a[:], w1xa[:], xls, start=True, stop=False)
                    nc.tensor.matmul(ps1a[:], w1za[:], zct[:], start=False, stop=True)
                    ps1b = psum.tile([128, BLK], f32, tag="ps1b")
                    nc.tensor.matmul(ps1b[:], w1xb[:], xls, start=True, stop=False)
                    nc.tensor.matmul(ps1b[:], w1zb[:], zct[:], start=False, stop=True)

                    # h1 halves stacked as the two DoubleRow k-tiles, fp8
                    h1 = pool.tile([128, 2 * BLK], fp8, tag="h1")
                    nc.scalar.activation(h1[:, 0:BLK], ps1a[:], Act.Relu,
                                         bias=b1a[:])
                    nc.scalar.activation(h1[:, BLK:2 * BLK], ps1b[:], Act.Relu,
                                         bias=b1b[:])
                    h1v = h1[:].rearrange("p (k c) -> p k c", k=2)

                    # the (2x-scaled) lo-term runs on even blocks only: the
                    # correction is ~3% of scale so 2x-on-half-the-cells is
                    # first-order exact through the relu and the segment mean
                    lo = blk % 2 == 0
                    ps2a = psum.tile([128, BLK], f32, tag="ps2a")
                    nc.tensor.matmul(ps2a[:], w2t[0, 0], h1v, start=True,
                                     stop=not lo, perf_mode=DR)
                    if lo:
                        nc.tensor.matmul(ps2a[:], w2t[0, 1], h1v, start=False,
                                         stop=True, perf_mode=DR)
                    ps2b = psum.tile([128, BLK], f32, tag="ps2b")
                    nc.tensor.matmul(ps2b[:], w2t[1, 0], h1v, start=True,
                                     stop=not lo, perf_mode=DR)
                    if lo:
                        nc.tensor.matmul(ps2b[:], w2t[1, 1], h1v, start=False,
                                         stop=True, perf_mode=DR)

                    h2 = pool.tile([128, 2 * BLK], bf16, tag="h2")
                    nc.vector.tensor_scalar(h2[:, 0:BLK], ps2a[:], b2a[:], 0.0,
                                            op0=Alu.add, op1=Alu.max)
                    nc.vector.tensor_scalar(h2[:, BLK:2 * BLK], ps2b[:], b2b[:],
                                            0.0, op0=Alu.add, op1=Alu.max)

                    # GpSimd pre-folds each 64-cell minichunk in half
                    # (SBUF->SBUF add), halving the DVE reduce read size.
                    h2v = h2[:].rearrange("p (g t m) -> p g t m", t=2, m=MC // 2)
                    h2f = pool.tile([128, BLK], bf16, tag="h2f")
                    h2fv = h2f[:].rearrange("p (g m) -> p g m", m=MC // 2)
                    nc.gpsimd.tensor_tensor(
                        h2fv, h2v[:, :, 0:1, :], h2v[:, :, 1:2, :], op=Alu.add)

                    oslice = slice(blk * 2 * (BLK // MC),
                                   (blk + 1) * 2 * (BLK // MC))
                    nc.vector.tensor_reduce(
                        out2[:, oslice], h2fv,
                        axis=mybir.AxisListType.X, op=Alu.add)
                if sblk + 2 < nsb:
                    xls_ahead.append(emit_ln(sblk + 2))

            nc.sync.dma_start(out_d[:], out2[:])

    nc.compile()
    return nc


def _get_program(nblk):
    if nblk not in _compiled:
        _compiled[nblk] = _build_program(nblk)
    return _compiled[nblk]


def kernel(X, Z, W1, b1, W2, b2, W3, b3, cell_to_batch, sample_idx_batch):
    X = np.asarray(X)
    Z = np.asarray(Z)
    W1 = np.asarray(W1, dtype=np.float32)
    b1 = np.asarray(b1, dtype=np.float32)
    W2 = np.asarray(W2, dtype=np.float32)
    b2 = np.asarray(b2, dtype=np.float32)
    W3 = np.asarray(W3, dtype=np.float32)
    b3 = np.asarray(b3, dtype=np.float32)
    c2b = np.asarray(cell_to_batch).astype(np.int64)
    sib = np.asarray(sample_idx_batch).astype(np.int64)

    n = X.shape[0]
    nseg = sib.shape[0]
    seg = sib[c2b]

    # ---- host layout prep -------------------------------------------------
    order = np.argsort(seg, kind="stable")
    seg_sorted = seg[order]
    counts = np.bincount(seg, minlength=nseg).astype(np.int64)
    padded = ((counts + MC - 1) // MC) * MC
    starts = np.concatenate([[0], np.cumsum(padded)])[:nseg]
    total_pad = int(padded.sum())
    nblk = NBLK
    while total_pad > N_CORES * nblk * BLK:  # safety fallback, recompiles
        nblk += 2
    ntot = N_CORES * nblk * BLK
    mc_per_core = nblk * (BLK // MC)
    run_starts = np.concatenate([[0], np.cumsum(counts)])[:nseg]
    ranks = np.arange(n, dtype=np.int64) - run_starts[seg_sorted]
    slots = starts[seg_sorted] + ranks

    Xs = np.zeros((ntot, DX), dtype=BF16)
    Xs[slots] = X[order].astype(BF16)
    Zs = np.zeros((ntot, DZ), dtype=BF16)
    Zs[slots] = Z[c2b[order]].astype(BF16)

    xt = np.ascontiguousarray(
        Xs.reshape(N_CORES, nblk // 2, 2 * BLK, DX).transpose(0, 1, 3, 2))
    zct = np.ascontiguousarray(
        Zs.reshape(N_CORES, nblk, BLK, DZ).transpose(0, 1, 3, 2))

    n_mc = ntot // MC
    mc_label = np.full(n_mc, -1, dtype=np.int64)
    mc_real = np.zeros(n_mc, dtype=np.int64)
    mc_of_slot = slots // MC
    mc_label[mc_of_slot] = seg_sorted
    np.add.at(mc_real, mc_of_slot, 1)

    # ---- weights ----------------------------------------------------------
    w1x = np.ascontiguousarray(W1[:DX]).astype(BF16)
    w1z = np.ascontiguousarray(W1[DX:DX + DZ]).astype(BF16)
    # W2 as a scaled fp8 (hi, lo) pair; together they are W2 to ~4e-4
    w2f = W2.astype(BF16).astype(np.float32) * W2SCALE
    t_hi = w2f.astype(FP8)
    # lo term ships pre-doubled: it is applied on even blocks only
    t_lo = (2.0 * (w2f - t_hi.astype(np.float32))).astype(FP8)
    w2q = np.zeros((2, 2, 128, 2 * 128), dtype=FP8)
    for m in range(2):
        for t, term in enumerate((t_hi, t_lo)):
            # [p, ktile*128] with element [p, k*128+mc] = term[k*128+p, m*128+mc]
            w2q[m, t] = (term.reshape(2, 128, H).transpose(1, 0, 2)
                         [:, :, m * 128:(m + 1) * 128].reshape(128, 256))
    b1d = np.ascontiguousarray(b1.reshape(2, 128, 1))
    b2d = np.ascontiguousarray(b2.reshape(2, 128, 1)) * W2SCALE

    # ---- run on 8 cores ---------------------------------------------------
    nc = _get_program(nblk)
    in_maps = []
    for c in range(N_CORES):
        in_maps.append({
            "xt": xt[c], "zct": zct[c],
            "w1x": w1x, "w1z": w1z, "w2": w2q, "b1": b1d, "b2": b2d,
        })
    global _last_in_maps
    _last_in_maps = in_maps
    res = run_bass_kernel_spmd(nc, in_maps, list(range(N_CORES)))

    # ---- host epilogue ----------------------------------------------------
    per_core = []
    for c in range(N_CORES):
        o = res.results[c]["out"].reshape(128, nblk, 2, BLK // MC)
        per_core.append(np.concatenate(
            [o[:, :, 0, :].reshape(128, mc_per_core),
             o[:, :, 1, :].reshape(128, mc_per_core)], axis=0))
    sums = np.concatenate(per_core, axis=1)  # [256, n_mc], scaled by W2SCALE

    # analytic contribution of one pad cell (X=0, Z=0), matching device math;
    # even blocks include the doubled lo-term, odd blocks are hi-only
    h1p = np.maximum(b1, 0.0).astype(FP8).astype(np.float32)
    w2eff = t_hi.astype(np.float32) + t_lo.astype(np.float32)
    v_pad_even = np.maximum(h1p @ w2eff + W2SCALE * b2, 0.0) \
        .astype(BF16).astype(np.float32)
    v_pad_odd = np.maximum(h1p @ t_hi.astype(np.float32) + W2SCALE * b2, 0.0) \
        .astype(BF16).astype(np.float32)
    mc_parity = (np.arange(n_mc) // (BLK // MC)) % 2
    v_pad = np.where(mc_parity[None, :] == 0,
                     v_pad_even[:, None], v_pad_odd[:, None])
    sums = sums - v_pad * (MC - mc_real).astype(np.float32)[None, :]
    sums /= W2SCALE

    valid = mc_label >= 0
    S = np.zeros((nseg, H), dtype=np.float32)
    np.add.at(S, mc_label[valid], sums[:, valid].T)

    denom = np.maximum(counts, 1).astype(np.float32)[:, None]
    Y = S @ W3 / denom + b3[None, :]
    Y[counts == 0] = 0.0
    return Y.astype(np.float32)



# revision 5
# speedup vs baseline: 1.4703x; 1.4703x over previous
"""Trainium2 Bass kernel for CompositionModel (gnn_message_passing).

Model: per-cell MLP over [log1p(X) ++ Z[cell_to_batch]] followed by a
segment-mean over batch labels.

Strategy (v2):
  * Host: log1p(X) precomputed and shipped fp8; cells sorted by segment and
    each segment padded to a multiple of 512 so every 512-cell block is
    single-segment; X blocks shipped twice (side by side) so one fp8
    DoubleRow matmul applies W1x_hi (k-tile 1) and W1x_lo (k-tile 2) -- W1
    is then effectively exact.  Z's contribution (Z @ W1z + b1) is a single
    per-block f32 bias vector applied by the ACT engine during relu1.
  * Device (8 cores, data-parallel over blocks, identical static program):
      L1: 2 fp8 DR matmuls -> PSUM; ACT relu1 (+zb bias, /64 descale) ->
      fp8 h1; L2: per m-half one DR matmul with W2_hi (full block) plus one
      DR matmul with 2*W2_lo on the first half of the columns (first-order
      exact through the segment mean); DVE tensor_scalar does
      bias+relu+cast AND the per-block segment sum via accum_out.
      The third (linear) layer commutes with the segment sum and is applied
      on the host to the 512x256 segment sums instead of 500k cells.
  * Host epilogue: subtract the analytically-known contribution of pad
    cells (xl=0 -> h1 = relu(zb)), combine block sums into segment sums,
    undo the x64 weight scale, apply W3/b3 and divide by true counts.
"""

import numpy as np
import ml_dtypes

import concourse.bacc as bacc
import concourse.mybir as mybir
import concourse.tile as tile
from concourse.bass_utils import run_bass_kernel_spmd

BF16 = ml_dtypes.bfloat16
FP8 = ml_dtypes.float8_e4m3fn

N_CORES = 8
DX = 128
DZ = 32
H = 256
B = 512
BLK = 512          # cells per block == segment pad quantum
SB = 4             # blocks per superblock (one DMA)
NBLK = 132         # blocks per core (fits the fixed reference input)
WSCALE = 64.0      # fp8 pre-scale on W1/W2/b2, divided out at the end

_compiled = {}
_last_in_maps = None


def _build_program(nblk):
    f32 = mybir.dt.float32
    bf16 = mybir.dt.bfloat16
    fp8 = mybir.dt.float8e4
    Alu = mybir.AluOpType
    Act = mybir.ActivationFunctionType
    DR = mybir.MatmulPerfMode.DoubleRow
    nsb = nblk // SB

    nc = bacc.Bacc("TRN2", target_bir_lowering=False, debug=False,
                   num_devices=N_CORES)

    # [super][p][xl0..xl3 | xl0..xl3]  (k-tile stride = SB*BLK)
    xt_d = nc.dram_tensor("xt", [nsb, DX, 2 * SB * BLK], fp8,
                          kind="ExternalInput")
    # [m-half][p, ktile*128] fp8: k1 = 64*W1x_hi, k2 = 64*W1x_lo
    w1_d = nc.dram_tensor("w1", [2, 128, 256], fp8, kind="ExternalInput")
    w2hi_d = nc.dram_tensor("w2hi", [2, 128, 256], fp8, kind="ExternalInput")
    w2lo_d = nc.dram_tensor("w2lo", [2, 128, 256], fp8, kind="ExternalInput")
    zb_d = nc.dram_tensor("zb", [128, 2 * nblk], f32, kind="ExternalInput")
    b2_d = nc.dram_tensor("b2", [2, 128, 1], f32, kind="ExternalInput")
    out_d = nc.dram_tensor("out", [128, 2 * nblk], f32, kind="ExternalOutput")

    with tile.TileContext(nc) as tc:
        with tc.tile_pool(name="consts", bufs=1) as cpool, \
             tc.tile_pool(name="xt", bufs=3) as xtpool, \
             tc.tile_pool(name="h1", bufs=3) as h1pool, \
             tc.tile_pool(name="hsc", bufs=2) as hscpool, \
             tc.tile_pool(name="psum", bufs=2, space="PSUM") as psum:

            w1t, w2hit, w2lot = [], [], []
            for h in range(2):
                for lst, src in ((w1t, w1_d), (w2hit, w2hi_d),
                                 (w2lot, w2lo_d)):
                    w = cpool.tile([128, 256], fp8, tag=f"w{len(lst)}_{id(src)}")
                    nc.sync.dma_start(w[:], src[h])
                    lst.append(w[:].rearrange("p (k m) -> p k m", k=2))
            zbt = cpool.tile([128, 2 * nblk], f32, tag="zbt")
            nc.sync.dma_start(zbt[:], zb_d[:])
            zeros = cpool.tile([128, 512], bf16, tag="zeros")
            nc.vector.memset(zeros[:], 0.0)
            b2a = cpool.tile([128, 1], f32, tag="b2a")
            b2b = cpool.tile([128, 1], f32, tag="b2b")
            nc.sync.dma_start(b2a[:], b2_d[0])
            nc.sync.dma_start(b2b[:], b2_d[1])
            outt = cpool.tile([128, 2 * nblk], f32, tag="outt")

            xt_tiles = {}
            ps1_tiles = {}
            h1_tiles = {}
            ps2_tiles = {}

            def load_super(j):
                if j >= nsb:
                    return
                t = xtpool.tile([DX, 2 * SB * BLK], fp8, tag="xt")
                nc.sync.dma_start(t[:], xt_d[j])
                xt_tiles[j] = t

            def emit_l1(i):
                j, b = divmod(i, SB)
                xt = xt_tiles[j]
                xv = xt[:].rearrange("p (k g c) -> p g k c", k=2, g=SB)[:, b]
                ps1 = psum.tile([128, 1024], f32, tag="ps1")
                nc.tensor.matmul(ps1[:, 0:512], w1t[0], xv,
                                 start=True, stop=True, perf_mode=DR)
                nc.tensor.matmul(ps1[:, 512:1024], w1t[1], xv,
                                 start=True, stop=True, perf_mode=DR)
                ps1_tiles[i] = ps1
                if b == SB - 1:
                    xt_tiles.pop(j)

            def emit_relu1(i):
                ps1 = ps1_tiles.pop(i)
                h1 = h1pool.tile([128, 1024], fp8, tag="h1")
                nc.scalar.activation(h1[:, 0:512], ps1[:, 0:512], Act.Relu,
                                     bias=zbt[:, i:i + 1],
                                     scale=1.0 / WSCALE)
                nc.scalar.activation(h1[:, 512:1024], ps1[:, 512:1024],
                                     Act.Relu,
                                     bias=zbt[:, nblk + i:nblk + i + 1],
                                     scale=1.0 / WSCALE)
                h1_tiles[i] = h1

            def emit_l2(i):
                h1 = h1_tiles.pop(i)
                h1v = h1[:].rearrange("p (k c) -> p k c", k=2)
                ps2 = psum.tile([128, 1024], f32, tag="ps2")
                for h in range(2):
                    o = h * 512
                    nc.tensor.matmul(ps2[:, o:o + 512], w2hit[h], h1v,
                                     start=True, stop=False, perf_mode=DR)
                    # 2x the lo-term on the first half of the cells:
                    # first-order exact through the segment mean
                    nc.tensor.matmul(ps2[:, o:o + 256], w2lot[h],
                                     h1v[:, :, 0:256], start=False, stop=True,
                                     perf_mode=DR, skip_group_check=True)
                ps2_tiles[i] = ps2

            def emit_relu2(i):
                ps2 = ps2_tiles.pop(i)
                for h, b2x in enumerate((b2a, b2b)):
                    hsc = hscpool.tile([128, 512], bf16, tag=f"hsc{h}")
                    # out = max(ps2 + b2, 0); accum_out = sum(out) over cells
                    nc.vector.scalar_tensor_tensor(
                        hsc[:], ps2[:, h * 512:(h + 1) * 512], b2x[:],
                        zeros[:], op0=Alu.add, op1=Alu.max,
                        accum_out=outt[:, h * nblk + i:h * nblk + i + 1])

            load_super(0)
            load_super(1)
            emit_l1(0)
            emit_relu1(0)
            for i in range(1, nblk):
                if i % SB == 0:
                    load_super(i // SB + 1)
                emit_l1(i)
                emit_relu1(i)
                emit_l2(i - 1)
                emit_relu2(i - 1)
            emit_l2(nblk - 1)
            emit_relu2(nblk - 1)

            nc.sync.dma_start(out_d[:], outt[:])

    nc.compile()
    return nc


def _get_program(nblk):
    if nblk not in _compiled:
        _compiled[nblk] = _build_program(nblk)
    return _compiled[nblk]


def _q8(x):
    return np.asarray(x, dtype=np.float32).astype(FP8)


def kernel(X, Z, W1, b1, W2, b2, W3, b3, cell_to_batch, sample_idx_batch):
    X = np.asarray(X)
    Z = np.asarray(Z, dtype=np.float32)
    W1 = np.asarray(W1, dtype=np.float32)
    b1 = np.asarray(b1, dtype=np.float32)
    W2 = np.asarray(W2, dtype=np.float32)
    b2 = np.asarray(b2, dtype=np.float32)
    W3 = np.asarray(W3, dtype=np.float32)
    b3 = np.asarray(b3, dtype=np.float32)
    c2b = np.asarray(cell_to_batch).astype(np.int64)
    sib = np.asarray(sample_idx_batch).astype(np.int64)

    n = X.shape[0]
    nseg = sib.shape[0]
    seg = sib[c2b]

    # ---- host layout prep -------------------------------------------------
    order = np.argsort(seg, kind="stable")
    seg_sorted = seg[order]
    counts = np.bincount(seg, minlength=nseg).astype(np.int64)
    padded = ((counts + BLK - 1) // BLK) * BLK
    starts = np.concatenate([[0], np.cumsum(padded)])[:nseg]
    total_pad = int(padded.sum())
    nblk = NBLK
    while total_pad > N_CORES * nblk * BLK:  # safety fallback, recompiles
        nblk += SB
    ntot = N_CORES * nblk * BLK
    nb_tot = ntot // BLK
    run_starts = np.concatenate([[0], np.cumsum(counts)])[:nseg]
    ranks = np.arange(n, dtype=np.int64) - run_starts[seg_sorted]
    slots = starts[seg_sorted] + ranks

    xl8 = _q8(np.log1p(np.asarray(X, dtype=np.float32)))
    Xs = np.zeros((ntot, DX), dtype=FP8)
    Xs[slots] = xl8[order]

    # [core, nblk, 128, 512] -> supers [core, nsb, 128, SB*512] doubled
    nsb = nblk // SB
    xtc = Xs.reshape(N_CORES, nsb, SB, BLK, DX).transpose(0, 1, 4, 2, 3)
    xtc = np.ascontiguousarray(xtc).reshape(N_CORES, nsb, DX, SB * BLK)
    xt = np.concatenate([xtc, xtc], axis=3)
    xt = np.ascontiguousarray(xt)

    # per-block segment labels / real-cell counts
    nb_of_seg = (padded // BLK).astype(np.int64)
    blk_label = np.full(nb_tot, -1, dtype=np.int64)
    fill = np.repeat(np.arange(nseg), nb_of_seg)
    blk_label[:fill.shape[0]] = fill
    blk_real = np.zeros(nb_tot, dtype=np.int64)
    blk_real[:fill.shape[0]] = BLK
    last_blk = (starts + padded) // BLK - 1
    blk_real[last_blk] = counts - (nb_of_seg - 1) * BLK

    # ---- weights ----------------------------------------------------------
    w1s = W1[:DX] * WSCALE                      # [128, 256]
    w1_hi = _q8(w1s)
    w1_lo = _q8(w1s - w1_hi.astype(np.float32))
    w2s = W2 * WSCALE                           # [256, 256]
    w2_hi = _q8(w2s)
    w2_lo2 = _q8(2.0 * (w2s - w2_hi.astype(np.float32)))

    # w1: k1 = hi, k2 = lo (same xl streamed through both k-tiles)
    w1q = np.zeros((2, 128, 256), dtype=FP8)
    for h in range(2):
        w1q[h, :, 0:128] = w1_hi[:, h * 128:(h + 1) * 128]
        w1q[h, :, 128:256] = w1_lo[:, h * 128:(h + 1) * 128]
    # w2: ktile t covers contraction rows t*128..t*128+127
    w2hiq = np.zeros((2, 128, 256), dtype=FP8)
    w2loq = np.zeros((2, 128, 256), dtype=FP8)
    for h in range(2):
        for t in range(2):
            w2hiq[h, :, t * 128:(t + 1) * 128] = \
                w2_hi[t * 128:(t + 1) * 128, h * 128:(h + 1) * 128]
            w2loq[h, :, t * 128:(t + 1) * 128] = \
                w2_lo2[t * 128:(t + 1) * 128, h * 128:(h + 1) * 128]

    zb = Z @ W1[DX:DX + DZ] + b1                # [B, 256] exact f32
    zb_blk = np.zeros((nb_tot, H), dtype=np.float32)
    lbl = blk_label >= 0
    zb_blk[lbl] = zb[blk_label[lbl]]
    # zb_d[core]: [128, 2*nblk] with col h*nblk+i = zb of block i, m-half h
    zbd = zb_blk.reshape(N_CORES, nblk, 2, 128).transpose(0, 3, 2, 1)
    zbd = np.ascontiguousarray(zbd).reshape(N_CORES, 128, 2 * nblk)
    b2d = np.ascontiguousarray((b2 * WSCALE).reshape(2, 128, 1))

    # ---- run on 8 cores ---------------------------------------------------
    nc = _get_program(nblk)
    in_maps = []
    for c in range(N_CORES):
        in_maps.append({
            "xt": xt[c], "w1": w1q, "w2hi": w2hiq, "w2lo": w2loq,
            "zb": zbd[c], "b2": b2d,
        })
    global _last_in_maps
    _last_in_maps = in_maps
    res = run_bass_kernel_spmd(nc, in_maps, list(range(N_CORES)))

    # ---- host epilogue ----------------------------------------------------
    per_core = []
    for c in range(N_CORES):
        o = res.results[c]["out"]               # [128, 2*nblk]
        per_core.append(np.stack([o[:, 0:nblk], o[:, nblk:2 * nblk]], axis=0))
    sums = np.concatenate(per_core, axis=2)     # [2, 128, nb_tot]
    sums = sums.transpose(2, 0, 1).reshape(nb_tot, H)  # [nb_tot, 256]

    # pad-cell contribution: xl = 0 -> h1 = fp8(relu(zb)); cols < 256 get
    # the doubled lo term, cols >= 256 are hi-only
    h1p = _q8(np.maximum(zb, 0.0)).astype(np.float32)          # [B, 256]
    w2hi_f = w2_hi.astype(np.float32)
    w2full_f = w2hi_f + w2_lo2.astype(np.float32)
    pre_lo = h1p @ w2full_f + WSCALE * b2
    pre_hi = h1p @ w2hi_f + WSCALE * b2
    h2p_lo = np.maximum(pre_lo, 0.0)                           # [B, 256]
    h2p_hi = np.maximum(pre_hi, 0.0)

    r = blk_real[lbl]
    n_lo = np.maximum(0, 256 - r).astype(np.float32)
    n_hi = (BLK - r).astype(np.float32) - n_lo
    labs = blk_label[lbl]
    corr = n_lo[:, None] * h2p_lo[labs] + n_hi[:, None] * h2p_hi[labs]

    S = np.zeros((nseg, H), dtype=np.float32)
    np.add.at(S, labs, sums[lbl] - corr)
    S /= WSCALE

    denom = np.maximum(counts, 1).astype(np.float32)[:, None]
    Y = S @ W3 / denom + b3[None, :]
    Y[counts == 0] = 0.0
    return Y.astype(np.float32)


# revision 12
# speedup vs baseline: 1.6673x; 1.1340x over previous
"""Trainium2 Bass kernel for CompositionModel (gnn_message_passing).

Model: per-cell MLP over [log1p(X) ++ Z[cell_to_batch]] followed by a
segment-mean over batch labels.

Strategy (v2):
  * Host: log1p(X) precomputed and shipped fp8; cells sorted by segment and
    each segment padded to a multiple of 512 so every 512-cell block is
    single-segment; X blocks shipped twice (side by side) so one fp8
    DoubleRow matmul applies W1x_hi (k-tile 1) and W1x_lo (k-tile 2) -- W1
    is then effectively exact.  Z's contribution (Z @ W1z + b1) is a single
    per-block f32 bias vector applied by the ACT engine during relu1.
  * Device (8 cores, data-parallel over blocks, identical static program):
      L1: 2 fp8 DR matmuls -> PSUM; ACT relu1 (+zb bias, /64 descale) ->
      fp8 h1; L2: per m-half one DR matmul with W2_hi (full block) plus one
      DR matmul with 2*W2_lo on the first half of the columns (first-order
      exact through the segment mean); DVE tensor_scalar does
      bias+relu+cast AND the per-block segment sum via accum_out.
      The third (linear) layer commutes with the segment sum and is applied
      on the host to the 512x256 segment sums instead of 500k cells.
  * Host epilogue: subtract the analytically-known contribution of pad
    cells (xl=0 -> h1 = relu(zb)), combine block sums into segment sums,
    undo the x64 weight scale, apply W3/b3 and divide by true counts.
"""

import numpy as np
import ml_dtypes

import concourse.bacc as bacc
import concourse.mybir as mybir
import concourse.tile as tile
from concourse.bass_utils import run_bass_kernel_spmd

BF16 = ml_dtypes.bfloat16
FP8 = ml_dtypes.float8_e4m3fn

N_CORES = 8
DX = 128
DZ = 32
H = 256
B = 512
BLK = 512          # cells per block == segment pad quantum
SB = 4             # blocks per superblock (one DMA)
NBLK = 132         # blocks per core (fits the fixed reference input)
WSCALE = 64.0      # fp8 pre-scale on W1/W2/b2, divided out at the end

_compiled = {}
_last_in_maps = None


def _build_program(nblk):
    f32 = mybir.dt.float32
    bf16 = mybir.dt.bfloat16
    fp8 = mybir.dt.float8e4
    Alu = mybir.AluOpType
    Act = mybir.ActivationFunctionType
    DR = mybir.MatmulPerfMode.DoubleRow
    nsb = nblk // SB

    nc = bacc.Bacc("TRN2", target_bir_lowering=False, debug=False,
                   num_devices=N_CORES)

    # [super][p][xl0..xl3 | xl0..xl3]  (k-tile stride = SB*BLK)
    xt_d = nc.dram_tensor("xt", [nsb, DX, 2 * SB * BLK], fp8,
                          kind="ExternalInput")
    # [m-half][p, ktile*128] fp8: k1 = 64*W1x_hi, k2 = 64*W1x_lo
    w1_d = nc.dram_tensor("w1", [2, 128, 256], fp8, kind="ExternalInput")
    w2hi_d = nc.dram_tensor("w2hi", [2, 128, 256], fp8, kind="ExternalInput")
    zb_d = nc.dram_tensor("zb", [128, 2 * nblk], f32, kind="ExternalInput")
    # per-(block, m-half) relu2 bias: 64*b2 minus the W2-quantization
    # mean-correction for the block's segment
    b2c_d = nc.dram_tensor("b2c", [128, 2 * nblk], f32, kind="ExternalInput")
    out_d = nc.dram_tensor("out", [128, 2 * nblk], f32, kind="ExternalOutput")

    with tile.TileContext(nc) as tc:
        with tc.tile_pool(name="consts", bufs=1) as cpool, \
             tc.tile_pool(name="xt", bufs=3) as xtpool, \
             tc.tile_pool(name="h1", bufs=3) as h1pool, \
             tc.tile_pool(name="hsc", bufs=2) as hscpool, \
             tc.tile_pool(name="psum", bufs=2, space="PSUM") as psum:

            w1t, w2hit = [], []
            for h in range(2):
                for lst, src in ((w1t, w1_d), (w2hit, w2hi_d)):
                    w = cpool.tile([128, 256], fp8, tag=f"w{len(lst)}_{id(src)}")
                    nc.sync.dma_start(w[:], src[h])
                    lst.append(w[:].rearrange("p (k m) -> p k m", k=2))
            zbt = cpool.tile([128, 2 * nblk], f32, tag="zbt")
            nc.sync.dma_start(zbt[:], zb_d[:])
            b2ct = cpool.tile([128, 2 * nblk], f32, tag="b2ct")
            nc.sync.dma_start(b2ct[:], b2c_d[:])
            zeros = cpool.tile([128, 512], bf16, tag="zeros")
            nc.vector.memset(zeros[:], 0.0)
            outt = cpool.tile([128, 2 * nblk], f32, tag="outt")

            xt_tiles = {}
            ps1_tiles = {}
            h1_tiles = {}
            ps2_tiles = {}

            def load_super(j):
                if j >= nsb:
                    return
                t = xtpool.tile([DX, 2 * SB * BLK], fp8, tag="xt")
                nc.sync.dma_start(t[:], xt_d[j])
                xt_tiles[j] = t

            def emit_l1(i):
                j, b = divmod(i, SB)
                xt = xt_tiles[j]
                xv = xt[:].rearrange("p (k g c) -> p g k c", k=2, g=SB)[:, b]
                ps1 = psum.tile([128, 1024], f32, tag="ps1")
                nc.tensor.matmul(ps1[:, 0:512], w1t[0], xv,
                                 start=True, stop=True, perf_mode=DR)
                nc.tensor.matmul(ps1[:, 512:1024], w1t[1], xv,
                                 start=True, stop=True, perf_mode=DR)
                ps1_tiles[i] = ps1
                if b == SB - 1:
                    xt_tiles.pop(j)

            def emit_relu1(i):
                ps1 = ps1_tiles.pop(i)
                h1 = h1pool.tile([128, 1024], fp8, tag="h1")
                nc.scalar.activation(h1[:, 0:512], ps1[:, 0:512], Act.Relu,
                                     bias=zbt[:, i:i + 1],
                                     scale=1.0 / WSCALE)
                nc.scalar.activation(h1[:, 512:1024], ps1[:, 512:1024],
                                     Act.Relu,
                                     bias=zbt[:, nblk + i:nblk + i + 1],
                                     scale=1.0 / WSCALE)
                h1_tiles[i] = h1

            def emit_l2(i):
                h1 = h1_tiles.pop(i)
                h1v = h1[:].rearrange("p (k c) -> p k c", k=2)
                ps2 = psum.tile([128, 1024], f32, tag="ps2")
                for h in range(2):
                    o = h * 512
                    nc.tensor.matmul(ps2[:, o:o + 512], w2hit[h], h1v,
                                     start=True, stop=True, perf_mode=DR)
                ps2_tiles[i] = ps2

            def emit_relu2(i):
                ps2 = ps2_tiles.pop(i)
                for h in range(2):
                    hsc = hscpool.tile([128, 512], bf16, tag=f"hsc{h}")
                    c = h * nblk + i
                    # out = max(ps2 + b2c, 0); accum_out = sum(out) over cells
                    nc.vector.scalar_tensor_tensor(
                        hsc[:], ps2[:, h * 512:(h + 1) * 512],
                        b2ct[:, c:c + 1], zeros[:], op0=Alu.add, op1=Alu.max,
                        accum_out=outt[:, c:c + 1])

            load_super(0)
            load_super(1)
            emit_l1(0)
            emit_relu1(0)
            for i in range(1, nblk):
                if i % SB == 0:
                    load_super(i // SB + 1)
                emit_l1(i)
                emit_relu1(i)
                emit_l2(i - 1)
                emit_relu2(i - 1)
            emit_l2(nblk - 1)
            emit_relu2(nblk - 1)

            nc.sync.dma_start(out_d[:], outt[:])

    nc.compile()
    return nc


def _get_program(nblk):
    if nblk not in _compiled:
        _compiled[nblk] = _build_program(nblk)
    return _compiled[nblk]


def _q8(x):
    return np.asarray(x, dtype=np.float32).astype(FP8)


def kernel(X, Z, W1, b1, W2, b2, W3, b3, cell_to_batch, sample_idx_batch):
    X = np.asarray(X)
    Z = np.asarray(Z, dtype=np.float32)
    W1 = np.asarray(W1, dtype=np.float32)
    b1 = np.asarray(b1, dtype=np.float32)
    W2 = np.asarray(W2, dtype=np.float32)
    b2 = np.asarray(b2, dtype=np.float32)
    W3 = np.asarray(W3, dtype=np.float32)
    b3 = np.asarray(b3, dtype=np.float32)
    c2b = np.asarray(cell_to_batch).astype(np.int64)
    sib = np.asarray(sample_idx_batch).astype(np.int64)

    n = X.shape[0]
    nseg = sib.shape[0]
    seg = sib[c2b]

    # ---- host layout prep -------------------------------------------------
    order = np.argsort(seg, kind="stable")
    seg_sorted = seg[order]
    counts = np.bincount(seg, minlength=nseg).astype(np.int64)
    padded = ((counts + BLK - 1) // BLK) * BLK
    starts = np.concatenate([[0], np.cumsum(padded)])[:nseg]
    total_pad = int(padded.sum())
    nblk = NBLK
    while total_pad > N_CORES * nblk * BLK:  # safety fallback, recompiles
        nblk += SB
    ntot = N_CORES * nblk * BLK
    nb_tot = ntot // BLK
    run_starts = np.concatenate([[0], np.cumsum(counts)])[:nseg]
    ranks = np.arange(n, dtype=np.int64) - run_starts[seg_sorted]
    slots = starts[seg_sorted] + ranks

    xl8 = _q8(np.log1p(np.asarray(X, dtype=np.float32)))
    Xs = np.zeros((ntot, DX), dtype=FP8)
    Xs[slots] = xl8[order]

    # [core, nblk, 128, 512] -> supers [core, nsb, 128, SB*512] doubled
    nsb = nblk // SB
    xtc = Xs.reshape(N_CORES, nsb, SB, BLK, DX).transpose(0, 1, 4, 2, 3)
    xtc = np.ascontiguousarray(xtc).reshape(N_CORES, nsb, DX, SB * BLK)
    xt = np.concatenate([xtc, xtc], axis=3)
    xt = np.ascontiguousarray(xt)

    # per-block segment labels / real-cell counts
    nb_of_seg = (padded // BLK).astype(np.int64)
    blk_label = np.full(nb_tot, -1, dtype=np.int64)
    fill = np.repeat(np.arange(nseg), nb_of_seg)
    blk_label[:fill.shape[0]] = fill
    blk_real = np.zeros(nb_tot, dtype=np.int64)
    blk_real[:fill.shape[0]] = BLK
    last_blk = (starts + padded) // BLK - 1
    blk_real[last_blk] = counts - (nb_of_seg - 1) * BLK

    # ---- weights ----------------------------------------------------------
    from scipy.special import erf

    w1s = W1[:DX] * WSCALE                      # [128, 256]
    w1_hi = _q8(w1s)
    w1_lo = _q8(w1s - w1_hi.astype(np.float32))
    w2s = W2 * WSCALE                           # [256, 256]
    w2_hi = _q8(w2s)

    # w1: k1 = hi, k2 = lo (same xl streamed through both k-tiles)
    w1q = np.zeros((2, 128, 256), dtype=FP8)
    for h in range(2):
        w1q[h, :, 0:128] = w1_hi[:, h * 128:(h + 1) * 128]
        w1q[h, :, 128:256] = w1_lo[:, h * 128:(h + 1) * 128]
    # w2: ktile t covers contraction rows t*128..t*128+127
    w2hiq = np.zeros((2, 128, 256), dtype=FP8)
    for h in range(2):
        for t in range(2):
            w2hiq[h, :, t * 128:(t + 1) * 128] = \
                w2_hi[t * 128:(t + 1) * 128, h * 128:(h + 1) * 128]

    zb = Z @ W1[DX:DX + DZ] + b1                # [B, 256] exact f32
    zb_blk = np.zeros((nb_tot, H), dtype=np.float32)
    lbl = blk_label >= 0
    zb_blk[lbl] = zb[blk_label[lbl]]
    # zb_d[core]: [128, 2*nblk] with col h*nblk+i = zb of block i, m-half h
    zbd = zb_blk.reshape(N_CORES, nblk, 2, 128).transpose(0, 3, 2, 1)
    zbd = np.ascontiguousarray(zbd).reshape(N_CORES, 128, 2 * nblk)

    # relu2 bias with the per-segment W2-quantization mean-correction:
    # b2c = 64*(b2 - E[h1|seg] @ (W2_hi/64 - W2)), E[h1|seg] from a Gaussian
    # closed form over the actual shipped-data column moments.
    xlf = xl8.astype(np.float32)
    mcol = xlf.mean(axis=0)                     # [128]
    vcol = xlf.var(axis=0)                      # [128]
    w1eff = (w1_hi.astype(np.float32) + w1_lo.astype(np.float32)) / WSCALE
    mu = mcol @ w1eff                           # [256]
    sig = np.sqrt(np.maximum(vcol @ (w1eff ** 2), 1e-12))
    muz = mu[None, :] + zb                      # [B, 256]
    u = muz / sig[None, :]
    Phi = 0.5 * (1.0 + erf(u / np.sqrt(2.0)))
    phi = np.exp(-0.5 * u * u) / np.sqrt(2.0 * np.pi)
    Eh1 = sig[None, :] * phi + muz * Phi        # [B, 256]
    dW2 = w2_hi.astype(np.float32) / WSCALE - W2
    b2c_seg = WSCALE * (b2[None, :] - Eh1 @ dW2)   # [B, 256]
    b2c_blk = np.broadcast_to((WSCALE * b2)[None, :],
                              (nb_tot, H)).copy().astype(np.float32)
    b2c_blk[lbl] = b2c_seg[blk_label[lbl]]
    b2cd = b2c_blk.reshape(N_CORES, nblk, 2, 128).transpose(0, 3, 2, 1)
    b2cd = np.ascontiguousarray(b2cd).reshape(N_CORES, 128, 2 * nblk)

    # ---- run on 8 cores ---------------------------------------------------
    nc = _get_program(nblk)
    in_maps = []
    for c in range(N_CORES):
        in_maps.append({
            "xt": xt[c], "w1": w1q, "w2hi": w2hiq,
            "zb": zbd[c], "b2c": b2cd[c],
        })
    global _last_in_maps
    _last_in_maps = in_maps
    res = run_bass_kernel_spmd(nc, in_maps, list(range(N_CORES)))

    # ---- host epilogue ----------------------------------------------------
    per_core = []
    for c in range(N_CORES):
        o = res.results[c]["out"]               # [128, 2*nblk]
        per_core.append(np.stack([o[:, 0:nblk], o[:, nblk:2 * nblk]], axis=0))
    sums = np.concatenate(per_core, axis=2)     # [2, 128, nb_tot]
    sums = sums.transpose(2, 0, 1).reshape(nb_tot, H)  # [nb_tot, 256]

    # pad-cell contribution: xl = 0 -> h1 = fp8(relu(zb))
    h1p = _q8(np.maximum(zb, 0.0)).astype(np.float32)          # [B, 256]
    pre = h1p @ w2_hi.astype(np.float32) + b2c_seg
    h2p = np.maximum(pre, 0.0)                                 # [B, 256]

    r = blk_real[lbl]
    labs = blk_label[lbl]
    corr = (BLK - r).astype(np.float32)[:, None] * h2p[labs]

    S = np.zeros((nseg, H), dtype=np.float32)
    np.add.at(S, labs, sums[lbl] - corr)
    S /= WSCALE

    denom = np.maximum(counts, 1).astype(np.float32)[:, None]
    Y = S @ W3 / denom + b3[None, :]
    Y[counts == 0] = 0.0
    return Y.astype(np.float32)
